# revision 20
# baseline (speedup 1.0000x reference)
"""Trainium2 Bass kernel for nn_EwaldBlock (gnn_message_passing).

Strategy: shard by GRAPH (B=32 graphs -> 4 per core, batch-contiguous), so the
per-graph structure factors are computed entirely on one core and no
collective is needed.

v3 restructure (from the 54us v2 kernel), driven by the HW trace:

  * trig via 2-op range reduction: ms = mod(turns+16, 1), am = |ms-0.5|
    (turns = k_dot_r/2pi precomputed on host, shipped fp16).  Sin LUT then
    yields NEGATED cos/sin: -cos = Sin(pi/2 - 2pi*am), -sin = Sin(2pi*ms-pi).
    The message is quadratic in trig, so the sign cancels exactly.
    This kills 2 DVE passes, the old 4-pass reduction, and all clamping.
  * sinc shipped once ([128, T*K] bf16, not duplicated); two 64-col muls.
  * sins interleaved with MLP1-L1 silus on ACT (both tables preloaded via
    dummies during the DMA window) -- ACT runs continuously instead of
    serializing sins -> silus.
  * trigT PSUM->SBUF copies moved from ACT (3.7us) to DVE tensor_copy
    (bf16 2x mode, ~1.2us).
  * input DMAs spread over 4 queues (sync/scalar hwdge + vector/gpsimd
    swdge): per-queue streaming rate is ~134 B/ns, so 2 queues serialized
    input until 16.6us in v2.  Critical tensors first on each queue.
  * fp16 turns instead of fp32 k_dot_r (-278KB DMA, validated: adds <0.2%
    trig error on top of the accepted bf16 trig rounding).
"""

from contextlib import ExitStack

import numpy as np
import ml_dtypes

import concourse.bass as bass
import concourse.tile as tile
from concourse import mybir
from concourse.bass_utils import run_bass_kernel_spmd
from concourse.masks import make_identity

BF16 = mybir.dt.bfloat16
F16 = mybir.dt.float16
F32 = mybir.dt.float32
I32 = mybir.dt.int32
AF = mybir.ActivationFunctionType
ALU = mybir.AluOpType

N_CORES = 8
D = 128
K = 64
TWO_K = 2 * K
LN_EPS = 1e-5
PI = float(np.pi)
SQRT_MAGIC = 0x1FBD1DF5    # sqrt bit-trick seed: bits(sqrt(x)) ~ (bits(x)>>1)+M

CONFIG = {
    "act_mode": "silu",    # "silu" (HW) | "sigmoid_mul" (CoreSim-compatible)
    "split_waits": True,   # walrus needs <=1 wait/inst; CoreSim can't run nops
}

TRACE = False            # set by test harness for profiling
LAST_EXEC_NS = None
LAST_RESULTS = None

_PROGRAM_CACHE = {}


# --------------------------------------------------------------------------
# device program
# --------------------------------------------------------------------------

def _pieces(w, maxw=512):
    p = 0
    while p < w:
        pw = min(maxw, w - p)
        yield p, pw
        p += pw


def _tile_groups(tt, size):
    out = []
    t = 0
    while t < tt:
        out.append((t, min(size, tt - t)))
        t += size
    return out


_SPLIT_TYPES = (
    "InstTensorTensor", "InstTensorScalarPtr", "InstTensorCopy",
    "InstReciprocal", "InstBNStats", "InstBNStatsAggregate",
    "InstActivation", "InstMemset", "InstIota", "InstTensorReduce",
    "InstMatmult", "InstLdweights", "InstTensorScalarAffineSelect",
    "InstCopyPredicated", "InstDMACopy", "InstDrain",
)


def _split_excess_waits(nc, limit=1):
    """walrus's per-instruction ISA structs hold few sync waits (the DVE
    TensorTensor struct rejects >1).  Move excess waits onto same-engine
    NoOps inserted immediately before the instruction."""
    n_id = 0
    for f in nc.m.functions:
        for bb in f.blocks:
            insts = bb.instructions
            out = []
            for inst in insts:
                si = inst.sync_info
                if (si is not None and si.on_wait
                        and len(si.on_wait) > limit
                        and type(inst).__name__ in _SPLIT_TYPES):
                    waits = list(si.on_wait)
                    extra, keep = waits[:-limit], waits[-limit:]
                    for wchunk in [extra[i:i + limit]
                                   for i in range(0, len(extra), limit)]:
                        nop = mybir.InstNoOp(name=f"I-waitnop-{n_id}")
                        n_id += 1
                        nop.engine = inst.engine
                        nop.sync_info = mybir.SyncInfo(
                            on_wait=list(wchunk), on_update=[])
                        out.append(nop)
                    inst.sync_info = mybir.SyncInfo(
                        on_wait=keep, on_update=list(si.on_update))
                out.append(inst)
            insts[:] = out
    return nc


def build_program(slot_T):
    """SPMD Bass program for per-core graph-slot tile counts slot_T."""
    slot_T = tuple(int(t) for t in slot_T)
    G = len(slot_T)
    TT = sum(slot_T)
    n_pad = 128 * TT
    assert G * 128 <= 512, "sf PSUM bank holds at most 4 graphs"

    kgroups = _tile_groups(TT, 6)     # sin/trig groups, interleave with L1
    mgroups = _tile_groups(TT, 4)     # MLP chunks (512 node-cols)

    act_silu = CONFIG["act_mode"] == "silu"

    nc = bass.Bass()

    xtbf_d = nc.declare_dram_parameter("xtbf", [D, n_pad], BF16, isOutput=False)
    trn_d = nc.declare_dram_parameter("trn", [128, TT * K], F16, isOutput=False)
    sinc_d = nc.declare_dram_parameter("sinc", [128, TT * K], BF16,
                                       isOutput=False)
    wa_d = nc.declare_dram_parameter("wa", [D, 2 * D], BF16, isOutput=False)
    wb_d = nc.declare_dram_parameter("wb", [D, 3 * D], BF16, isOutput=False)
    out_d = nc.declare_dram_parameter("outt", [D, n_pad], BF16, isOutput=True)

    with tile.TileContext(nc) as tc, ExitStack() as ctx:
        consts = ctx.enter_context(tc.tile_pool(name="consts", bufs=1))
        pers = ctx.enter_context(tc.tile_pool(name="pers", bufs=1))
        work = ctx.enter_context(tc.tile_pool(name="work", bufs=4))
        ps = ctx.enter_context(tc.tile_pool(name="ps", bufs=4, space="PSUM"))
        trps = ctx.enter_context(tc.tile_pool(name="trps", bufs=2,
                                              space="PSUM"))
        sfps = ctx.enter_context(tc.tile_pool(name="sfps", bufs=1,
                                              space="PSUM"))

        # ---- input DMAs: 3 queues, deadline-ordered per queue -------------
        # Input DMA is aggregate-BW bound (~260 B/ns over all queues), so x
        # ships in ONE layout (feature-major) and the node-major copy is
        # rebuilt on-chip by PE transposes during the otherwise-idle DMA
        # window.  xtbf is split in thirds across the two hwdge queues.
        wa = consts.tile([D, 2 * D], BF16)
        nc.sync.dma_start(out=wa, in_=wa_d[:, :])
        xtbf = pers.tile([D, n_pad], BF16)
        third = max(512, ((n_pad // 3) // 256) * 256)
        cA, cB = third, min(2 * third, n_pad)
        nc.sync.dma_start(out=xtbf[:, 0:cA], in_=xtbf_d[:, 0:cA])
        nc.sync.dma_start(out=xtbf[:, cA:cB], in_=xtbf_d[:, cA:cB])

        trn_f = pers.tile([128, TT * K], F16)
        nc.scalar.dma_start(out=trn_f, in_=trn_d[:, :])
        if cB < n_pad:
            nc.scalar.dma_start(out=xtbf[:, cB:n_pad], in_=xtbf_d[:, cB:n_pad])
        wb = consts.tile([D, 3 * D], BF16)
        nc.scalar.dma_start(out=wb, in_=wb_d[:, :])

        sinc_f = pers.tile([128, TT * K], BF16)
        nc.gpsimd.dma_start(out=sinc_f, in_=sinc_d[:, :])

        xnm_f = pers.tile([128, TT * D], BF16)
        xnm = xnm_f.rearrange("p (t d) -> p t d", d=D)
        trn = trn_f.rearrange("p (t k) -> p t k", k=K)
        sinc = sinc_f.rearrange("p (t k) -> p t k", k=K)

        # ---- constants ---------------------------------------------------
        for i, cv in enumerate([0.0, PI / 2.0]):
            cvt = consts.tile([128, 1], F32, name=f"constap{i}")
            nc.gpsimd.memset(cvt, cv)
            nc.const_aps.aps[(F32, float(cv))] = cvt
        zcol = nc.const_aps.aps[(F32, 0.0)]

        ident = consts.tile([D, D], BF16)
        make_identity(nc, ident)

        def act(dst, src_psum):
            if act_silu:
                nc.scalar.activation(dst, src_psum, AF.Silu)
            else:
                sg = work.tile(list(dst.shape), BF16, name="sgm", tag="sgm")
                nc.scalar.activation(sg, src_psum, AF.Sigmoid)
                nc.vector.tensor_mul(dst, src_psum, sg)

        # preload both ACT tables while input DMAs are in flight
        dsin = consts.tile([128, 1], BF16)
        nc.scalar.activation(dsin, zcol, AF.Sin)
        if act_silu:
            dsil = consts.tile([128, 1], BF16)
            nc.scalar.activation(dsil, zcol, AF.Silu)

        # ---- range reduction (4 DVE ops per group, fp16) -----------------
        # frac = turns - round(turns) in [-0.5, 0.5] via the +1536 fp16
        # round-to-nearest trick (mod/abs_max are not valid HW TS ALU ops).
        # Emitted per trig group so the first sins start right after the
        # first slice of rr instead of after the full-width pass.
        RN16 = 1536.0
        aa_f = pers.tile([128, TT * K], F16)
        negn_f = pers.tile([128, TT * K], F16)
        fr_f = pers.tile([128, TT * K], F16)
        af_f = pers.tile([128, TT * K], F16)
        fr = fr_f.rearrange("p (t k) -> p t k", k=K)
        af = af_f.rearrange("p (t k) -> p t k", k=K)

        # ---- trig: cos = Sin(pi/2 - 2pi*|frac|), sin = Sin(2pi*frac) -----
        trig_f = pers.tile([128, TT * TWO_K], BF16)
        trig = trig_f.rearrange("p (t k) -> p t k", k=TWO_K)
        s1 = pers.tile([D, n_pad], BF16)

        def emit_trig_group(t0, nt):
            s = slice(K * t0, K * (t0 + nt))
            nc.vector.tensor_scalar(out=aa_f[:, s], in0=trn_f[:, s],
                                    scalar1=RN16, scalar2=None, op0=ALU.add)
            nc.vector.tensor_scalar(out=negn_f[:, s], in0=aa_f[:, s],
                                    scalar1=RN16, scalar2=-1.0,
                                    op0=ALU.subtract, op1=ALU.mult)
            nc.vector.tensor_add(fr_f[:, s], trn_f[:, s], negn_f[:, s])
            nc.vector.tensor_scalar(out=af_f[:, s].bitcast(mybir.dt.int16),
                                    in0=fr_f[:, s].bitcast(mybir.dt.int16),
                                    scalar1=0x7FFF, scalar2=None,
                                    op0=ALU.bitwise_and)
            cs = work.tile([128, nt, TWO_K], BF16, tag="cs", name=f"cs{t0}")
            nc.scalar.activation(cs[:, :, 0:K], af[:, t0:t0 + nt, :], AF.Sin,
                                 bias=PI / 2.0, scale=-2.0 * PI)
            nc.scalar.activation(cs[:, :, K:TWO_K], fr[:, t0:t0 + nt, :],
                                 AF.Sin, scale=2.0 * PI)
            nc.vector.tensor_mul(trig[:, t0:t0 + nt, 0:K], cs[:, :, 0:K],
                                 sinc[:, t0:t0 + nt, :])
            nc.vector.tensor_mul(trig[:, t0:t0 + nt, K:TWO_K],
                                 cs[:, :, K:TWO_K], sinc[:, t0:t0 + nt, :])

        def emit_l1_chunk(t0, nt):
            c0, w = 128 * t0, 128 * nt
            h1p = ps.tile([D, 512], F32, name=f"h1p{t0}", tag="ps")
            nc.tensor.matmul(h1p[:, 0:w], wa[:, 0:D], xtbf[:, c0:c0 + w],
                             start=True, stop=True)
            act(s1[:, c0:c0 + w], h1p[:, 0:w])

        def emit_xnm_chunk(t0, nt):
            """node-major x tiles via PE transpose of xtbf (DMA-idle window)."""
            xtp = trps.tile([128, 512], BF16, name=f"xtp{t0}", tag="tr")
            for i in range(nt):
                nc.tensor.transpose(xtp[:, i * 128:(i + 1) * 128],
                                    xtbf[:, 128 * (t0 + i):128 * (t0 + i + 1)],
                                    ident)
            nc.vector.tensor_copy(xnm_f[:, D * t0:D * (t0 + nt)],
                                  xtp[:, 0:128 * nt])

        trigT = pers.tile([TWO_K, n_pad], BF16)

        # ---- MLP1 layer 2 (node-major out) + residual + stats ------------
        xres_f = pers.tile([128, TT * D], BF16)
        xres = xres_f.rearrange("p (t d) -> p t d", d=D)
        stats = pers.tile([128, TT, 6], F32)
        xln_f = pers.tile([128, TT * D], BF16)
        xln = xln_f.rearrange("p (t d) -> p t d", d=D)
        mu = pers.tile([128, TT], F32)
        dd = pers.tile([128, TT], F32)
        cc = pers.tile([128, TT], F32)
        var = pers.tile([128, TT], F32)
        iv = pers.tile([128, TT], F32)
        rstd = pers.tile([128, TT], F32)
        t1 = pers.tile([128, TT], F32)

        def emit_mm2_chunk(t0, nt):
            c0, w = 128 * t0, 128 * nt
            h2p = ps.tile([128, 512], F32, name=f"h2p{t0}", tag="ps")
            for i in range(nt):
                nc.tensor.matmul(h2p[:, i * 128:(i + 1) * 128],
                                 s1[:, c0 + i * 128:c0 + (i + 1) * 128],
                                 wa[:, D:2 * D], start=True, stop=True)
            h2 = work.tile([128, 512], BF16, tag="h2", name=f"h2{t0}")
            act(h2[:, 0:w], h2p[:, 0:w])
            h2v = h2.rearrange("p (t d) -> p t d", d=D)
            nc.vector.tensor_add(xres[:, t0:t0 + nt, :],
                                 xnm[:, t0:t0 + nt, :], h2v[:, 0:nt, :])
            for i in range(nt):
                nc.vector.bn_stats(stats[:, t0 + i, :], xres[:, t0 + i, :])

        def emit_ln(a, b, mid=None):
            """mean + rstd (Newton rsqrt, no Sqrt table) + xln, tiles a:b.
            xln tiles [a:mid) go on DVE, [mid:b) on GpSimd (parallel)."""
            if mid is None:
                mid = b
            s = slice(a, b)
            m_e, m_o = stats[:, s, 1], stats[:, s, 4]
            cv_e, cv_o = stats[:, s, 2], stats[:, s, 5]
            nc.vector.tensor_add(mu[:, s], m_e, m_o)        # 2*mean
            nc.vector.tensor_scalar(out=mu[:, s], in0=mu[:, s], scalar1=0.5,
                                    scalar2=None, op0=ALU.mult)
            nc.vector.tensor_sub(dd[:, s], m_e, m_o)
            nc.vector.tensor_add(cc[:, s], cv_e, cv_o)
            nc.vector.tensor_scalar(out=cc[:, s], in0=cc[:, s],
                                    scalar1=1.0 / 128.0, scalar2=LN_EPS,
                                    op0=ALU.mult, op1=ALU.add)
            nc.vector.tensor_mul(dd[:, s], dd[:, s], dd[:, s])
            nc.vector.scalar_tensor_tensor(out=var[:, s], in0=dd[:, s],
                                           scalar=0.25, in1=cc[:, s],
                                           op0=ALU.mult, op1=ALU.add)
            nc.vector.reciprocal(iv[:, s], var[:, s])
            nc.vector.tensor_scalar(out=rstd[:, s].bitcast(I32),
                                    in0=iv[:, s].bitcast(I32),
                                    scalar1=1, scalar2=None,
                                    op0=ALU.arith_shift_right)
            nc.vector.tensor_scalar(out=rstd[:, s].bitcast(I32),
                                    in0=rstd[:, s].bitcast(I32),
                                    scalar1=SQRT_MAGIC, scalar2=None,
                                    op0=ALU.add)
            nc.vector.tensor_mul(t1[:, s], var[:, s], rstd[:, s])
            nc.vector.tensor_mul(t1[:, s], t1[:, s], rstd[:, s])
            nc.vector.tensor_scalar(out=t1[:, s], in0=t1[:, s], scalar1=-0.5,
                                    scalar2=1.5, op0=ALU.mult, op1=ALU.add)
            nc.vector.tensor_mul(rstd[:, s], rstd[:, s], t1[:, s])
            for t in range(a, b):
                eng = nc.vector if t < mid else nc.gpsimd
                eng.tensor_scalar(out=xln[:, t, :], in0=xres[:, t, :],
                                  scalar1=mu[:, t:t + 1],
                                  scalar2=rstd[:, t:t + 1],
                                  op0=ALU.subtract, op1=ALU.mult)

        slot_off = [0]
        for tj in slot_T:
            slot_off.append(slot_off[-1] + tj)
        kfr = wb[:, 2 * D:3 * D]
        sfp = sfps.tile([TWO_K, 512], F32, name="sfp", tag="sf")
        x2bf = pers.tile([D, n_pad], BF16)
        outb = pers.tile([D, n_pad], BF16)
        mlp2_done = [0]

        def emit_sf_msg(j):
            """SF accumulation + srsi + message matmul + x2 for graph j."""
            s0, Tj = slot_off[j], slot_T[j]
            for i in range(Tj):
                t = s0 + i
                nc.tensor.matmul(sfp[:, j * 128:j * 128 + D],
                                 trig[:, t, :], xln[:, t, :],
                                 start=(i == 0), stop=(i == Tj - 1))
            srsi = work.tile([TWO_K, D], BF16, tag="srsi", bufs=G,
                             name=f"srsi{j}")
            nc.vector.tensor_mul(srsi, sfp[:, j * 128:j * 128 + D], kfr)
            off = 128 * s0
            for p, pw in _pieces(128 * Tj):
                mg = ps.tile([D, 512], F32, name=f"mg{j}_{p}", tag="ps")
                nc.tensor.matmul(mg[:, 0:pw], srsi,
                                 trigT[:, off + p:off + p + pw],
                                 start=True, stop=True)
                nc.vector.tensor_add(x2bf[:, off + p:off + p + pw],
                                     xtbf[:, off + p:off + p + pw],
                                     mg[:, 0:pw])

        def emit_mlp2_ready(covered_cols):
            """MLP2 chunks whose x2bf columns are fully written."""
            while mlp2_done[0] < len(mgroups):
                t0, nt = mgroups[mlp2_done[0]]
                c0, w = 128 * t0, 128 * nt
                if c0 + w > covered_cols:
                    return
                u1p = ps.tile([D, 512], F32, name=f"u1p{t0}", tag="ps")
                nc.tensor.matmul(u1p[:, 0:w], wb[:, 0:D], x2bf[:, c0:c0 + w],
                                 start=True, stop=True)
                u1 = work.tile([D, 512], BF16, tag="u1", name=f"u1{t0}")
                act(u1[:, 0:w], u1p[:, 0:w])
                u2p = ps.tile([D, 512], F32, name=f"u2p{t0}", tag="ps")
                nc.tensor.matmul(u2p[:, 0:w], wb[:, D:2 * D], u1[:, 0:w],
                                 start=True, stop=True)
                u2 = work.tile([D, 512], BF16, tag="u2", name=f"u2{t0}")
                act(u2[:, 0:w], u2p[:, 0:w])
                # early chunks' residual adds go to the idle GpSimd; the
                # last two stay on DVE (GpSimd is ~2x slower per col and
                # would stretch the tail)
                eng = (nc.gpsimd if mlp2_done[0] < len(mgroups) - 2
                       else nc.vector)
                eng.tensor_add(outb[:, c0:c0 + w], x2bf[:, c0:c0 + w],
                               u2[:, 0:w])
                nc.sync.dma_start(out=out_d[:, c0:c0 + w],
                                  in_=outb[:, c0:c0 + w])
                mlp2_done[0] += 1

        def emit_trig_tr(t0, nt):
            trp = trps.tile([TWO_K, 512], BF16, name=f"trp{t0}", tag="tr")
            for i in range(nt):
                nc.tensor.transpose(trp[:, i * 128:(i + 1) * 128],
                                    trig[:, t0 + i, :], ident)
            nc.vector.tensor_copy(trigT[:, 128 * t0:128 * (t0 + nt)],
                                  trp[:, 0:128 * nt])

        # Front wave: sins + L1 + xnm transposes, with each L2 chunk woven
        # one-behind its L1 chunk so the first L2 silu (which gates the LN
        # chain) lands early in the ACT stream.
        for i in range(max(len(kgroups), len(mgroups)) + 1):
            if i < len(kgroups):
                emit_trig_group(*kgroups[i])
            if i < len(mgroups):
                emit_l1_chunk(*mgroups[i])
                emit_xnm_chunk(*mgroups[i])
            if 1 <= i <= len(mgroups):
                emit_mm2_chunk(*mgroups[i - 1])

        # LN in two halves (split at a graph-slot boundary) so the first
        # graphs' SF/MSG and MLP2 chunks overlap the second half's LN work.
        # xln per half splits DVE/GpSimd at the inner slot boundary.
        g_half = (G + 1) // 2
        t_half = slot_off[g_half]                   # tile where half 2 starts
        for (t0, nt) in mgroups:
            emit_trig_tr(t0, nt)
        emit_ln(0, t_half, mid=slot_off[1])
        for j in range(g_half):
            emit_sf_msg(j)
        emit_ln(t_half, TT, mid=slot_off[g_half + 1])
        emit_mlp2_ready(128 * slot_off[g_half])
        for j in range(g_half, G):
            emit_sf_msg(j)
            emit_mlp2_ready(128 * slot_off[j + 1])
        emit_mlp2_ready(n_pad)

    if CONFIG["split_waits"]:
        _split_excess_waits(nc)
    return nc


# --------------------------------------------------------------------------
# host side
# --------------------------------------------------------------------------

def _shard(batch, n_graphs):
    """Graph segments + serpentine graph->core/slot assignment."""
    bounds = np.searchsorted(batch, np.arange(n_graphs + 1))
    sizes = np.diff(bounds)
    order = np.argsort(-sizes, kind="stable")
    g_per_core = n_graphs // N_CORES
    gid = np.empty((N_CORES, g_per_core), dtype=np.int64)
    for j in range(g_per_core):
        sl = order[j * N_CORES:(j + 1) * N_CORES]
        if j % 2 == 1:
            sl = sl[::-1]
        gid[:, j] = sl
    slot_T = tuple(
        max(1, int(np.ceil(max(sizes[gid[c][j]] for c in range(N_CORES)) / 128)))
        for j in range(g_per_core))
    return bounds, gid, slot_T


def kernel(x_scalar, k_dot_r, sinc_damping, batch, down_projection,
           W_pre1, W_pre2, ln_gamma, ln_beta, W_up, W_upd1, W_upd2):
    x_scalar = np.asarray(x_scalar, dtype=np.float32)
    k_dot_r = np.asarray(k_dot_r, dtype=np.float32)
    sinc_damping = np.asarray(sinc_damping, dtype=np.float32)
    batch = np.asarray(batch).astype(np.int64)
    down_projection = np.asarray(down_projection, dtype=np.float32)
    W_pre1 = np.asarray(W_pre1, dtype=np.float32)
    W_pre2 = np.asarray(W_pre2, dtype=np.float32)
    ln_gamma = np.asarray(ln_gamma, dtype=np.float32)
    ln_beta = np.asarray(ln_beta, dtype=np.float32)
    W_up = np.asarray(W_up, dtype=np.float32)
    W_upd1 = np.asarray(W_upd1, dtype=np.float32)
    W_upd2 = np.asarray(W_upd2, dtype=np.float32)

    assert np.allclose(ln_beta, 0.0), "nonzero ln_beta not supported"

    n, d = x_scalar.shape
    n_graphs = int(batch.max()) + 1 if batch.size else 1
    n_graphs = max(n_graphs, N_CORES)
    while n_graphs % N_CORES:
        n_graphs += 1

    bounds, gid, slot_T = _shard(batch, n_graphs)
    g_per_core = n_graphs // N_CORES
    TT = sum(slot_T)
    n_pad = 128 * TT
    offs = np.cumsum([0] + [128 * t for t in slot_T])

    key = (slot_T, CONFIG["act_mode"], CONFIG["split_waits"])
    if key not in _PROGRAM_CACHE:
        _PROGRAM_CACHE[key] = build_program(slot_T)
    nc = _PROGRAM_CACHE[key]

    bf = ml_dtypes.bfloat16
    # kfilter with gamma folded, replicated for the cos and sin halves
    kf = down_projection @ (W_up * ln_gamma[:, None]).T        # [K, D]
    kfr = np.concatenate([kf, kf], axis=0)                     # [2K, D]
    shared = {
        "wa": np.ascontiguousarray(
            np.concatenate([W_pre1.T, W_pre2.T], axis=1)).astype(bf),
        "wb": np.ascontiguousarray(
            np.concatenate([W_upd1.T, W_upd2.T, kfr], axis=1)).astype(bf),
    }

    in_maps = []
    for c in range(N_CORES):
        xp = np.zeros((n_pad, D), np.float32)
        trnp = np.zeros((n_pad, K), np.float32)
        sincp = np.zeros((n_pad, K), np.float32)
        for j in range(g_per_core):
            g = gid[c][j]
            s, e = bounds[g], bounds[g + 1]
            xp[offs[j]:offs[j] + e - s] = x_scalar[s:e]
            trnp[offs[j]:offs[j] + e - s] = (
                k_dot_r[s:e] * np.float32(1.0 / (2.0 * np.pi)))
            sincp[offs[j]:offs[j] + e - s] = sinc_damping[s:e]

        # node-major [n_pad, F] -> per-tile [128, T*F] shuffled layout
        def shuf(a):
            f = a.shape[1]
            blk = np.transpose(a.reshape(TT, 128, f), (1, 0, 2))
            return np.ascontiguousarray(blk.reshape(128, TT * f))

        xt = np.ascontiguousarray(xp.T)
        in_maps.append(dict(shared,
                            xtbf=xt.astype(bf),
                            trn=shuf(trnp).astype(np.float16),
                            sinc=shuf(sincp).astype(bf)))

    global LAST_EXEC_NS, LAST_RESULTS
    res = run_bass_kernel_spmd(nc, in_maps, list(range(N_CORES)), trace=TRACE)
    LAST_RESULTS = res
    LAST_EXEC_NS = getattr(res, "exec_time_ns", None)
    out = np.zeros((n, d), np.float32)
    for c in range(N_CORES):
        outT = np.asarray(res.results[c]["outt"]).astype(np.float32)
        for j in range(g_per_core):
            g = gid[c][j]
            s, e = bounds[g], bounds[g + 1]
            out[s:e] = outT[:, offs[j]:offs[j] + e - s].T
    return out


# revision 24
# speedup vs baseline: 1.1449x; 1.1449x over previous
"""Trainium2 Bass kernel for nn_EwaldBlock (gnn_message_passing).

Strategy: shard by GRAPH (B=32 graphs -> 4 per core, batch-contiguous), so the
per-graph structure factors are computed entirely on one core and no
collective is needed.

v3 restructure (from the 54us v2 kernel), driven by the HW trace:

  * trig via 2-op range reduction: ms = mod(turns+16, 1), am = |ms-0.5|
    (turns = k_dot_r/2pi precomputed on host, shipped fp16).  Sin LUT then
    yields NEGATED cos/sin: -cos = Sin(pi/2 - 2pi*am), -sin = Sin(2pi*ms-pi).
    The message is quadratic in trig, so the sign cancels exactly.
    This kills 2 DVE passes, the old 4-pass reduction, and all clamping.
  * sinc shipped once ([128, T*K] bf16, not duplicated); two 64-col muls.
  * sins interleaved with MLP1-L1 silus on ACT (both tables preloaded via
    dummies during the DMA window) -- ACT runs continuously instead of
    serializing sins -> silus.
  * trigT PSUM->SBUF copies moved from ACT (3.7us) to DVE tensor_copy
    (bf16 2x mode, ~1.2us).
  * input DMAs spread over 4 queues (sync/scalar hwdge + vector/gpsimd
    swdge): per-queue streaming rate is ~134 B/ns, so 2 queues serialized
    input until 16.6us in v2.  Critical tensors first on each queue.
  * fp16 turns instead of fp32 k_dot_r (-278KB DMA, validated: adds <0.2%
    trig error on top of the accepted bf16 trig rounding).
"""

from contextlib import ExitStack

import numpy as np
import ml_dtypes

import concourse.bass as bass
import concourse.tile as tile
from concourse import mybir
from concourse.bass_utils import run_bass_kernel_spmd
from concourse.masks import make_identity

BF16 = mybir.dt.bfloat16
F16 = mybir.dt.float16
F32 = mybir.dt.float32
I32 = mybir.dt.int32
AF = mybir.ActivationFunctionType
ALU = mybir.AluOpType

N_CORES = 8
D = 128
K = 64
TWO_K = 2 * K
LN_EPS = 1e-5
PI = float(np.pi)
SQRT_MAGIC = 0x1FBD1DF5    # sqrt bit-trick seed: bits(sqrt(x)) ~ (bits(x)>>1)+M

CONFIG = {
    "act_mode": "silu",    # "silu" (HW) | "sigmoid_mul" (CoreSim-compatible)
    "split_waits": True,   # walrus needs <=1 wait/inst; CoreSim can't run nops
}

TRACE = False            # set by test harness for profiling
LAST_EXEC_NS = None
LAST_RESULTS = None

_PROGRAM_CACHE = {}


# --------------------------------------------------------------------------
# device program
# --------------------------------------------------------------------------

def _pieces(w, maxw=512):
    p = 0
    while p < w:
        pw = min(maxw, w - p)
        yield p, pw
        p += pw


def _tile_groups(tt, size):
    out = []
    t = 0
    while t < tt:
        out.append((t, min(size, tt - t)))
        t += size
    return out


_SPLIT_TYPES = (
    "InstTensorTensor", "InstTensorScalarPtr", "InstTensorCopy",
    "InstReciprocal", "InstBNStats", "InstBNStatsAggregate",
    "InstActivation", "InstMemset", "InstIota", "InstTensorReduce",
    "InstMatmult", "InstLdweights", "InstTensorScalarAffineSelect",
    "InstCopyPredicated", "InstDMACopy", "InstDrain",
)


def _split_excess_waits(nc, limit=1):
    """walrus's per-instruction ISA structs hold few sync waits (the DVE
    TensorTensor struct rejects >1).  Move excess waits onto same-engine
    NoOps inserted immediately before the instruction."""
    n_id = 0
    for f in nc.m.functions:
        for bb in f.blocks:
            insts = bb.instructions
            out = []
            for inst in insts:
                si = inst.sync_info
                if (si is not None and si.on_wait
                        and len(si.on_wait) > limit
                        and type(inst).__name__ in _SPLIT_TYPES):
                    waits = list(si.on_wait)
                    extra, keep = waits[:-limit], waits[-limit:]
                    for wchunk in [extra[i:i + limit]
                                   for i in range(0, len(extra), limit)]:
                        nop = mybir.InstNoOp(name=f"I-waitnop-{n_id}")
                        n_id += 1
                        nop.engine = inst.engine
                        nop.sync_info = mybir.SyncInfo(
                            on_wait=list(wchunk), on_update=[])
                        out.append(nop)
                    inst.sync_info = mybir.SyncInfo(
                        on_wait=keep, on_update=list(si.on_update))
                out.append(inst)
            insts[:] = out
    return nc


def build_program(slot_T):
    """SPMD Bass program for per-core graph-slot tile counts slot_T."""
    slot_T = tuple(int(t) for t in slot_T)
    G = len(slot_T)
    TT = sum(slot_T)
    n_pad = 128 * TT
    assert G * 128 <= 512, "sf PSUM bank holds at most 4 graphs"

    kgroups = _tile_groups(TT, 6)     # sin/trig groups, interleave with L1
    mgroups = _tile_groups(TT, 4)     # MLP chunks (512 node-cols)

    act_silu = CONFIG["act_mode"] == "silu"

    nc = bass.Bass()

    xtbf_d = nc.declare_dram_parameter("xtbf", [D, n_pad], BF16, isOutput=False)
    trn_d = nc.declare_dram_parameter("trn", [128, TT * K], F16, isOutput=False)
    sinc_d = nc.declare_dram_parameter("sinc", [128, TT * K], BF16,
                                       isOutput=False)
    wa_d = nc.declare_dram_parameter("wa", [D, 2 * D], BF16, isOutput=False)
    wb_d = nc.declare_dram_parameter("wb", [D, 3 * D], BF16, isOutput=False)
    out_d = nc.declare_dram_parameter("outt", [D, n_pad], BF16, isOutput=True)

    with tile.TileContext(nc) as tc, ExitStack() as ctx:
        consts = ctx.enter_context(tc.tile_pool(name="consts", bufs=1))
        pers = ctx.enter_context(tc.tile_pool(name="pers", bufs=1))
        work = ctx.enter_context(tc.tile_pool(name="work", bufs=4))
        ps = ctx.enter_context(tc.tile_pool(name="ps", bufs=4, space="PSUM"))
        trps = ctx.enter_context(tc.tile_pool(name="trps", bufs=2,
                                              space="PSUM"))
        sfps = ctx.enter_context(tc.tile_pool(name="sfps", bufs=1,
                                              space="PSUM"))

        # ---- input DMAs: 3 queues, deadline-ordered per queue -------------
        # Input DMA is aggregate-BW bound (~260 B/ns over all queues), so x
        # ships in ONE layout (feature-major) and the node-major copy is
        # rebuilt on-chip by PE transposes during the otherwise-idle DMA
        # window.  xtbf is split in thirds across the two hwdge queues.
        wa = consts.tile([D, 2 * D], BF16)
        nc.sync.dma_start(out=wa, in_=wa_d[:, :])
        xtbf = pers.tile([D, n_pad], BF16)
        third = max(512, ((n_pad // 3) // 256) * 256)
        cA, cB = third, min(2 * third, n_pad)
        nc.sync.dma_start(out=xtbf[:, 0:cA], in_=xtbf_d[:, 0:cA])
        nc.sync.dma_start(out=xtbf[:, cA:cB], in_=xtbf_d[:, cA:cB])

        trn_f = pers.tile([128, TT * K], F16)
        nc.scalar.dma_start(out=trn_f, in_=trn_d[:, :])
        if cB < n_pad:
            nc.scalar.dma_start(out=xtbf[:, cB:n_pad], in_=xtbf_d[:, cB:n_pad])
        wb = consts.tile([D, 3 * D], BF16)
        nc.scalar.dma_start(out=wb, in_=wb_d[:, :])

        sinc_f = pers.tile([128, TT * K], BF16)
        nc.gpsimd.dma_start(out=sinc_f, in_=sinc_d[:, :])

        xnm_f = pers.tile([128, TT * D], BF16)
        xnm = xnm_f.rearrange("p (t d) -> p t d", d=D)
        trn = trn_f.rearrange("p (t k) -> p t k", k=K)
        sinc = sinc_f.rearrange("p (t k) -> p t k", k=K)

        # ---- constants ---------------------------------------------------
        for i, cv in enumerate([0.0, PI / 2.0]):
            cvt = consts.tile([128, 1], F32, name=f"constap{i}")
            nc.gpsimd.memset(cvt, cv)
            nc.const_aps.aps[(F32, float(cv))] = cvt
        zcol = nc.const_aps.aps[(F32, 0.0)]

        ident = consts.tile([D, D], BF16)
        make_identity(nc, ident)

        def act(dst, src_psum):
            if act_silu:
                nc.scalar.activation(dst, src_psum, AF.Silu)
            else:
                sg = work.tile(list(dst.shape), BF16, name="sgm", tag="sgm")
                nc.scalar.activation(sg, src_psum, AF.Sigmoid)
                nc.vector.tensor_mul(dst, src_psum, sg)

        # preload the ACT tables while input DMAs are in flight
        dsin = consts.tile([128, 1], BF16)
        nc.scalar.activation(dsin, zcol, AF.Sin)
        if act_silu:
            dsil = consts.tile([128, 1], BF16)
            nc.scalar.activation(dsil, zcol, AF.Silu)
        dsqr = consts.tile([128, 1], F32)
        nc.scalar.activation(dsqr, zcol, AF.Sqrt)

        # ---- range reduction (4 DVE ops per group, fp16) -----------------
        # frac = turns - round(turns) in [-0.5, 0.5] via the +1536 fp16
        # round-to-nearest trick (mod/abs_max are not valid HW TS ALU ops).
        # Emitted per trig group so the first sins start right after the
        # first slice of rr instead of after the full-width pass.
        RN16 = 1536.0
        aa_f = pers.tile([128, TT * K], F16)
        negn_f = pers.tile([128, TT * K], F16)
        fr_f = pers.tile([128, TT * K], F16)
        af_f = pers.tile([128, TT * K], F16)
        fr = fr_f.rearrange("p (t k) -> p t k", k=K)
        af = af_f.rearrange("p (t k) -> p t k", k=K)

        # ---- trig: cos = Sin(pi/2 - 2pi*|frac|), sin = Sin(2pi*frac) -----
        trig_f = pers.tile([128, TT * TWO_K], BF16)
        trig = trig_f.rearrange("p (t k) -> p t k", k=TWO_K)
        s1 = pers.tile([D, n_pad], BF16)

        def emit_trig_group(t0, nt):
            s = slice(K * t0, K * (t0 + nt))
            nc.vector.tensor_scalar(out=aa_f[:, s], in0=trn_f[:, s],
                                    scalar1=RN16, scalar2=None, op0=ALU.add)
            nc.vector.tensor_scalar(out=negn_f[:, s], in0=aa_f[:, s],
                                    scalar1=RN16, scalar2=-1.0,
                                    op0=ALU.subtract, op1=ALU.mult)
            nc.vector.tensor_add(fr_f[:, s], trn_f[:, s], negn_f[:, s])
            nc.vector.tensor_scalar(out=af_f[:, s].bitcast(mybir.dt.int16),
                                    in0=fr_f[:, s].bitcast(mybir.dt.int16),
                                    scalar1=0x7FFF, scalar2=None,
                                    op0=ALU.bitwise_and)
            cs = work.tile([128, nt, TWO_K], BF16, tag="cs", name=f"cs{t0}")
            nc.scalar.activation(cs[:, :, 0:K], af[:, t0:t0 + nt, :], AF.Sin,
                                 bias=PI / 2.0, scale=-2.0 * PI)
            nc.scalar.activation(cs[:, :, K:TWO_K], fr[:, t0:t0 + nt, :],
                                 AF.Sin, scale=2.0 * PI)
            nc.vector.tensor_mul(trig[:, t0:t0 + nt, 0:K], cs[:, :, 0:K],
                                 sinc[:, t0:t0 + nt, :])
            nc.vector.tensor_mul(trig[:, t0:t0 + nt, K:TWO_K],
                                 cs[:, :, K:TWO_K], sinc[:, t0:t0 + nt, :])

        def emit_l1_chunk(t0, nt):
            c0, w = 128 * t0, 128 * nt
            h1p = ps.tile([D, 512], F32, name=f"h1p{t0}", tag="ps")
            nc.tensor.matmul(h1p[:, 0:w], wa[:, 0:D], xtbf[:, c0:c0 + w],
                             start=True, stop=True)
            act(s1[:, c0:c0 + w], h1p[:, 0:w])

        def emit_xnm_chunk(t0, nt):
            """node-major x tiles via PE transpose of xtbf (DMA-idle window)."""
            xtp = trps.tile([128, 512], BF16, name=f"xtp{t0}", tag="tr")
            for i in range(nt):
                nc.tensor.transpose(xtp[:, i * 128:(i + 1) * 128],
                                    xtbf[:, 128 * (t0 + i):128 * (t0 + i + 1)],
                                    ident)
            nc.vector.tensor_copy(xnm_f[:, D * t0:D * (t0 + nt)],
                                  xtp[:, 0:128 * nt])

        trigT = pers.tile([TWO_K, n_pad], BF16)

        # ---- MLP1 layer 2 (node-major out) + residual + stats ------------
        xres_f = pers.tile([128, TT * D], BF16)
        xres = xres_f.rearrange("p (t d) -> p t d", d=D)
        stats = pers.tile([128, TT, 6], F32)
        xln_f = pers.tile([128, TT * D], BF16)
        xln = xln_f.rearrange("p (t d) -> p t d", d=D)
        mu = pers.tile([128, TT], F32)
        dd = pers.tile([128, TT], F32)
        cc = pers.tile([128, TT], F32)
        var = pers.tile([128, TT], F32)
        iv = pers.tile([128, TT], F32)
        rstd = pers.tile([128, TT], F32)
        t1 = pers.tile([128, TT], F32)

        def emit_mm2_chunk(t0, nt):
            c0, w = 128 * t0, 128 * nt
            h2p = ps.tile([128, 512], F32, name=f"h2p{t0}", tag="ps")
            for i in range(nt):
                nc.tensor.matmul(h2p[:, i * 128:(i + 1) * 128],
                                 s1[:, c0 + i * 128:c0 + (i + 1) * 128],
                                 wa[:, D:2 * D], start=True, stop=True)
            h2 = work.tile([128, 512], BF16, tag="h2", name=f"h2{t0}")
            act(h2[:, 0:w], h2p[:, 0:w])
            h2v = h2.rearrange("p (t d) -> p t d", d=D)
            nc.vector.tensor_add(xres[:, t0:t0 + nt, :],
                                 xnm[:, t0:t0 + nt, :], h2v[:, 0:nt, :])
            for i in range(nt):
                nc.vector.bn_stats(stats[:, t0 + i, :], xres[:, t0 + i, :])

        def emit_ln(a, b, mid=None):
            """mean + rstd (Newton rsqrt, no Sqrt table) + xln, tiles a:b.
            xln tiles [a:mid) go on DVE, [mid:b) on GpSimd (parallel)."""
            if mid is None:
                mid = b
            s = slice(a, b)
            m_e, m_o = stats[:, s, 1], stats[:, s, 4]
            cv_e, cv_o = stats[:, s, 2], stats[:, s, 5]
            nc.vector.tensor_add(mu[:, s], m_e, m_o)        # 2*mean
            nc.vector.tensor_scalar(out=mu[:, s], in0=mu[:, s], scalar1=0.5,
                                    scalar2=None, op0=ALU.mult)
            nc.vector.tensor_sub(dd[:, s], m_e, m_o)
            nc.vector.tensor_add(cc[:, s], cv_e, cv_o)
            nc.vector.tensor_scalar(out=cc[:, s], in0=cc[:, s],
                                    scalar1=1.0 / 128.0, scalar2=LN_EPS,
                                    op0=ALU.mult, op1=ALU.add)
            nc.vector.tensor_mul(dd[:, s], dd[:, s], dd[:, s])
            nc.vector.scalar_tensor_tensor(out=var[:, s], in0=dd[:, s],
                                           scalar=0.25, in1=cc[:, s],
                                           op0=ALU.mult, op1=ALU.add)
            nc.vector.reciprocal(iv[:, s], var[:, s])
            nc.scalar.activation(rstd[:, s], iv[:, s], AF.Sqrt)
            for t in range(a, b):
                nc.vector.tensor_scalar(out=xln[:, t, :], in0=xres[:, t, :],
                                        scalar1=mu[:, t:t + 1],
                                        scalar2=rstd[:, t:t + 1],
                                        op0=ALU.subtract, op1=ALU.mult)

        slot_off = [0]
        for tj in slot_T:
            slot_off.append(slot_off[-1] + tj)
        kfr = wb[:, 2 * D:3 * D]
        sfp = sfps.tile([TWO_K, 512], F32, name="sfp", tag="sf")
        x2bf = pers.tile([D, n_pad], BF16)
        outb = pers.tile([D, n_pad], BF16)
        mlp2_done = [0]

        def emit_sf_msg(j):
            """SF accumulation + srsi + message matmul + x2 for graph j."""
            s0, Tj = slot_off[j], slot_T[j]
            for i in range(Tj):
                t = s0 + i
                nc.tensor.matmul(sfp[:, j * 128:j * 128 + D],
                                 trig[:, t, :], xln[:, t, :],
                                 start=(i == 0), stop=(i == Tj - 1))
            srsi = work.tile([TWO_K, D], BF16, tag="srsi", bufs=G,
                             name=f"srsi{j}")
            nc.vector.tensor_mul(srsi, sfp[:, j * 128:j * 128 + D], kfr)
            off = 128 * s0
            for p, pw in _pieces(128 * Tj):
                mg = ps.tile([D, 512], F32, name=f"mg{j}_{p}", tag="ps")
                nc.tensor.matmul(mg[:, 0:pw], srsi,
                                 trigT[:, off + p:off + p + pw],
                                 start=True, stop=True)
                nc.vector.tensor_add(x2bf[:, off + p:off + p + pw],
                                     xtbf[:, off + p:off + p + pw],
                                     mg[:, 0:pw])

        def emit_mlp2_ready(covered_cols):
            """MLP2 chunks whose x2bf columns are fully written."""
            while mlp2_done[0] < len(mgroups):
                t0, nt = mgroups[mlp2_done[0]]
                c0, w = 128 * t0, 128 * nt
                if c0 + w > covered_cols:
                    return
                u1p = ps.tile([D, 512], F32, name=f"u1p{t0}", tag="ps")
                nc.tensor.matmul(u1p[:, 0:w], wb[:, 0:D], x2bf[:, c0:c0 + w],
                                 start=True, stop=True)
                u1 = work.tile([D, 512], BF16, tag="u1", name=f"u1{t0}")
                act(u1[:, 0:w], u1p[:, 0:w])
                u2p = ps.tile([D, 512], F32, name=f"u2p{t0}", tag="ps")
                nc.tensor.matmul(u2p[:, 0:w], wb[:, D:2 * D], u1[:, 0:w],
                                 start=True, stop=True)
                u2 = work.tile([D, 512], BF16, tag="u2", name=f"u2{t0}")
                act(u2[:, 0:w], u2p[:, 0:w])
                nc.vector.tensor_add(outb[:, c0:c0 + w], x2bf[:, c0:c0 + w],
                                     u2[:, 0:w])
                nc.sync.dma_start(out=out_d[:, c0:c0 + w],
                                  in_=outb[:, c0:c0 + w])
                mlp2_done[0] += 1

        def emit_trig_tr(t0, nt):
            trp = trps.tile([TWO_K, 512], BF16, name=f"trp{t0}", tag="tr")
            for i in range(nt):
                nc.tensor.transpose(trp[:, i * 128:(i + 1) * 128],
                                    trig[:, t0 + i, :], ident)
            nc.vector.tensor_copy(trigT[:, 128 * t0:128 * (t0 + nt)],
                                  trp[:, 0:128 * nt])

        # Front wave: sins + L1 + xnm transposes, with each L2 chunk woven
        # one-behind its L1 chunk so the first L2 silu (which gates the LN
        # chain) lands early in the ACT stream.
        for i in range(max(len(kgroups), len(mgroups)) + 1):
            if i < len(kgroups):
                emit_trig_group(*kgroups[i])
            if i < len(mgroups):
                emit_l1_chunk(*mgroups[i])
                emit_xnm_chunk(*mgroups[i])
            if 1 <= i <= len(mgroups):
                emit_mm2_chunk(*mgroups[i - 1])

        # LN in two halves (split at a graph-slot boundary) so the first
        # graphs' SF/MSG and MLP2 chunks overlap the second half's LN work.
        # xln per half splits DVE/GpSimd at the inner slot boundary.
        g_half = (G + 1) // 2
        t_half = slot_off[g_half]                   # tile where half 2 starts
        for (t0, nt) in mgroups:
            emit_trig_tr(t0, nt)
        emit_ln(0, t_half, mid=slot_off[1])
        for j in range(g_half):
            emit_sf_msg(j)
        emit_ln(t_half, TT, mid=slot_off[g_half + 1])
        emit_mlp2_ready(128 * slot_off[g_half])
        for j in range(g_half, G):
            emit_sf_msg(j)
            emit_mlp2_ready(128 * slot_off[j + 1])
        emit_mlp2_ready(n_pad)

    if CONFIG["split_waits"]:
        _split_excess_waits(nc)
    return nc


# --------------------------------------------------------------------------
# host side
# --------------------------------------------------------------------------

def _shard(batch, n_graphs):
    """Graph segments + serpentine graph->core/slot assignment."""
    bounds = np.searchsorted(batch, np.arange(n_graphs + 1))
    sizes = np.diff(bounds)
    order = np.argsort(-sizes, kind="stable")
    g_per_core = n_graphs // N_CORES
    gid = np.empty((N_CORES, g_per_core), dtype=np.int64)
    for j in range(g_per_core):
        sl = order[j * N_CORES:(j + 1) * N_CORES]
        if j % 2 == 1:
            sl = sl[::-1]
        gid[:, j] = sl
    slot_T = tuple(
        max(1, int(np.ceil(max(sizes[gid[c][j]] for c in range(N_CORES)) / 128)))
        for j in range(g_per_core))
    return bounds, gid, slot_T


def kernel(x_scalar, k_dot_r, sinc_damping, batch, down_projection,
           W_pre1, W_pre2, ln_gamma, ln_beta, W_up, W_upd1, W_upd2):
    x_scalar = np.asarray(x_scalar, dtype=np.float32)
    k_dot_r = np.asarray(k_dot_r, dtype=np.float32)
    sinc_damping = np.asarray(sinc_damping, dtype=np.float32)
    batch = np.asarray(batch).astype(np.int64)
    down_projection = np.asarray(down_projection, dtype=np.float32)
    W_pre1 = np.asarray(W_pre1, dtype=np.float32)
    W_pre2 = np.asarray(W_pre2, dtype=np.float32)
    ln_gamma = np.asarray(ln_gamma, dtype=np.float32)
    ln_beta = np.asarray(ln_beta, dtype=np.float32)
    W_up = np.asarray(W_up, dtype=np.float32)
    W_upd1 = np.asarray(W_upd1, dtype=np.float32)
    W_upd2 = np.asarray(W_upd2, dtype=np.float32)

    assert np.allclose(ln_beta, 0.0), "nonzero ln_beta not supported"

    n, d = x_scalar.shape
    n_graphs = int(batch.max()) + 1 if batch.size else 1
    n_graphs = max(n_graphs, N_CORES)
    while n_graphs % N_CORES:
        n_graphs += 1

    bounds, gid, slot_T = _shard(batch, n_graphs)
    g_per_core = n_graphs // N_CORES
    TT = sum(slot_T)
    n_pad = 128 * TT
    offs = np.cumsum([0] + [128 * t for t in slot_T])

    key = (slot_T, CONFIG["act_mode"], CONFIG["split_waits"])
    if key not in _PROGRAM_CACHE:
        _PROGRAM_CACHE[key] = build_program(slot_T)
    nc = _PROGRAM_CACHE[key]

    bf = ml_dtypes.bfloat16
    # kfilter with gamma folded, replicated for the cos and sin halves
    kf = down_projection @ (W_up * ln_gamma[:, None]).T        # [K, D]
    kfr = np.concatenate([kf, kf], axis=0)                     # [2K, D]
    shared = {
        "wa": np.ascontiguousarray(
            np.concatenate([W_pre1.T, W_pre2.T], axis=1)).astype(bf),
        "wb": np.ascontiguousarray(
            np.concatenate([W_upd1.T, W_upd2.T, kfr], axis=1)).astype(bf),
    }

    in_maps = []
    for c in range(N_CORES):
        xp = np.zeros((n_pad, D), np.float32)
        trnp = np.zeros((n_pad, K), np.float32)
        sincp = np.zeros((n_pad, K), np.float32)
        for j in range(g_per_core):
            g = gid[c][j]
            s, e = bounds[g], bounds[g + 1]
            xp[offs[j]:offs[j] + e - s] = x_scalar[s:e]
            trnp[offs[j]:offs[j] + e - s] = (
                k_dot_r[s:e] * np.float32(1.0 / (2.0 * np.pi)))
            sincp[offs[j]:offs[j] + e - s] = sinc_damping[s:e]

        # node-major [n_pad, F] -> per-tile [128, T*F] shuffled layout
        def shuf(a):
            f = a.shape[1]
            blk = np.transpose(a.reshape(TT, 128, f), (1, 0, 2))
            return np.ascontiguousarray(blk.reshape(128, TT * f))

        xt = np.ascontiguousarray(xp.T)
        in_maps.append(dict(shared,
                            xtbf=xt.astype(bf),
                            trn=shuf(trnp).astype(np.float16),
                            sinc=shuf(sincp).astype(bf)))

    global LAST_EXEC_NS, LAST_RESULTS
    res = run_bass_kernel_spmd(nc, in_maps, list(range(N_CORES)), trace=TRACE)
    LAST_RESULTS = res
    LAST_EXEC_NS = getattr(res, "exec_time_ns", None)
    out = np.zeros((n, d), np.float32)
    for c in range(N_CORES):
        outT = np.asarray(res.results[c]["outt"]).astype(np.float32)
        for j in range(g_per_core):
            g = gid[c][j]
            s, e = bounds[g], bounds[g + 1]
            out[s:e] = outT[:, offs[j]:offs[j] + e - s].T
    return out
